# revision 5
# baseline (speedup 1.0000x reference)
"""Trainium2 Bass/Tile kernel for ExtAttentionPool.

Math (per sample b):
    S[u, o]  = sum_d L[u, d] * W[o, d]                     (scores*O, pre bias/scale)
    E[o, u]  = exp(0.1 * S[u, o] + 0.1 * b[o])             (softmax numerator over u)
    Z[o]     = sum_u E[o, u]
    OUT[o,t] = (1/Z[o]) * sum_c E[o, c] * L[t, c]
    result row = OUT flattened (O-major), shape (O*T,)

Sharding: data-parallel over batch B=16 across 8 cores (2 samples/core).

Implementation notes:
  - Both matmuls contract over logits' D axis, so logits is transposed
    on-chip via PE transpose-mode (128x128 tiles), staged through PSUM,
    copied to SBUF (copies split between DVE and ACT engines).
  - Matmuls run as float32r (1 cycle/row on TRN2 for free dim >= 256).
  - Softmax uses ScalarE Exp with accum_out to get Z in the same pass;
    the 1/Z scaling is folded into the final PSUM->SBUF copy.
"""

import numpy as np
from contextlib import ExitStack

import concourse.bass as bass
import concourse.mybir as mybir
import concourse.tile as tile
from concourse import bacc
from concourse.bass_utils import run_bass_kernel_spmd
from concourse.masks import make_identity

F32 = mybir.dt.float32
F32R = mybir.dt.float32r

N_CORES = 8
B_FULL = 16


def build_nc(b_per=2, T=1024, D=1024, O=10, transpose_f32r=False):
    """Build the per-core Bass program. Same program on all 8 cores."""
    P = 128
    NT = T // P          # t-chunks
    ND = D // P          # d-chunks
    NH = max(1, T // 512)  # output free-dim halves (512 wide)
    HW = min(T, 512)     # half width

    nc = bacc.Bacc(
        "TRN2", target_bir_lowering=False, debug=False, enable_asserts=False
    )
    logits = nc.dram_tensor("logits", (b_per, T, D), F32, kind="ExternalInput").ap()
    w_in = nc.dram_tensor("W", (O, D), F32, kind="ExternalInput").ap()
    b_in = nc.dram_tensor("b", (O,), F32, kind="ExternalInput").ap()
    out = nc.dram_tensor("out", (b_per, O * T), F32, kind="ExternalOutput").ap()

    tdt = F32R if transpose_f32r else F32

    with tile.TileContext(nc) as tc, ExitStack() as ctx:
        singles = ctx.enter_context(tc.tile_pool(name="singles", bufs=1))
        lr_pool = ctx.enter_context(tc.tile_pool(name="lr", bufs=3))
        lt_pool = ctx.enter_context(tc.tile_pool(name="lt", bufs=2))
        e_pool = ctx.enter_context(tc.tile_pool(name="e", bufs=2))
        z_pool = ctx.enter_context(tc.tile_pool(name="z", bufs=2))
        osb_pool = ctx.enter_context(tc.tile_pool(name="osb", bufs=2))
        slab_ps = ctx.enter_context(tc.tile_pool(name="slab", bufs=2, space="PSUM"))
        s_ps = ctx.enter_context(tc.tile_pool(name="sps", bufs=2, space="PSUM"))
        o_ps = ctx.enter_context(tc.tile_pool(name="ops", bufs=2, space="PSUM"))
        et_ps = ctx.enter_context(tc.tile_pool(name="etps", bufs=2, space="PSUM"))

        # --- constants / weights prep (once) ---
        ident = singles.tile([P, P], F32)
        make_identity(nc, ident)

        w_sb = singles.tile([O, D], F32)
        nc.sync.dma_start(out=w_sb, in_=w_in)
        b_sb = singles.tile([O, 1], F32)
        nc.sync.dma_start(out=b_sb, in_=b_in.rearrange("(o u) -> o u", u=1))
        bias01 = singles.tile([O, 1], F32)
        # bias01 = 0.1 * b
        nc.scalar.activation(
            out=bias01, in_=b_sb, func=mybir.ActivationFunctionType.Copy, scale=1.0 / O
        )

        # WT[dp, c, o] = W[o, 128c + dp], via PE transpose of W rows
        wt_stage = et_ps.tile([P, ND, O], F32, tag="etps")
        for c in range(ND):
            nc.tensor.transpose(
                wt_stage[:, c, :], w_sb[:, c * P : (c + 1) * P], ident[:O, :O]
            )
        wt_sb = singles.tile([P, ND, O], F32R)
        nc.vector.tensor_copy(wt_sb, wt_stage)

        def phase_load_transpose(s):
            """DMA logits[s] in and build LT[dp, c, t] = L[t, 128c+dp]."""
            lt = lt_pool.tile([P, ND, T], F32R, tag="lt")
            gsz = min(4, ND)  # transposes per PSUM slab (4 fills one bank)
            for r in range(NT):
                lr = lr_pool.tile([P, D], F32, tag="lr")
                nc.sync.dma_start(out=lr, in_=logits[s, r * P : (r + 1) * P, :])
                for g in range(ND // gsz):
                    slab = slab_ps.tile([P, gsz * P], F32, tag="slab")
                    for k in range(gsz):
                        c = gsz * g + k
                        if transpose_f32r:
                            nc.tensor.transpose(
                                slab[:, k * P : (k + 1) * P].bitcast(F32R),
                                lr[:, c * P : (c + 1) * P].bitcast(F32R),
                                ident.bitcast(F32R),
                            )
                        else:
                            nc.tensor.transpose(
                                slab[:, k * P : (k + 1) * P],
                                lr[:, c * P : (c + 1) * P],
                                ident,
                            )
                    # copy slab -> LT[:, g*gsz:(g+1)*gsz, r*P:(r+1)*P]
                    dst = lt[:, gsz * g : gsz * (g + 1), r * P : (r + 1) * P]
                    if g % 2 == 0:
                        nc.vector.tensor_copy(dst, slab)
                    else:
                        nc.scalar.activation(
                            out=dst, in_=slab,
                            func=mybir.ActivationFunctionType.Copy,
                        )
            return lt

        def phase_mm1(s, lt):
            """S^T accumulation: s_tiles[h] (O, HW) psum."""
            s_tiles = []
            for h in range(NH):
                sp = s_ps.tile([O, HW], F32, tag="sps")
                for c in range(ND):
                    nc.tensor.matmul(
                        sp,
                        lhsT=wt_sb[:, c, :],
                        rhs=lt[:, c, h * HW : (h + 1) * HW],
                        start=(c == 0),
                        stop=(c == ND - 1),
                    )
                s_tiles.append(sp)
            return s_tiles

        def phase_softmax(s, s_tiles):
            """E = exp(0.1*S + 0.1*b) with accumulated Z; returns (E, rZ)."""
            e_sb = e_pool.tile([O, T], F32, tag="e")
            zparts = z_pool.tile([O, NH], F32, tag="z")
            for h in range(NH):
                nc.scalar.activation(
                    out=e_sb[:, h * HW : (h + 1) * HW],
                    in_=s_tiles[h],
                    func=mybir.ActivationFunctionType.Exp,
                    scale=1.0 / O,
                    bias=bias01,
                    accum_out=zparts[:, h : h + 1],
                )
            zsum = z_pool.tile([O, 1], F32, tag="zs")
            if NH == 2:
                nc.vector.tensor_add(zsum, zparts[:, 0:1], zparts[:, 1:2])
            elif NH == 1:
                nc.vector.tensor_copy(zsum, zparts)
            else:
                nc.vector.reduce_sum(zsum, zparts, axis=mybir.AxisListType.X)
            rz = z_pool.tile([O, 1], F32, tag="rz")
            nc.vector.reciprocal(rz, zsum)
            return e_sb, rz

        def phase_et(s, e_sb):
            """EC[cp, c, o] = E[o, 128c+cp] via PE transposes."""
            et_stage = et_ps.tile([P, ND, O], F32, tag="etps")
            for c in range(ND):
                nc.tensor.transpose(
                    et_stage[:, c, :], e_sb[:, c * P : (c + 1) * P], ident[:O, :O]
                )
            ec = e_pool.tile([P, ND, O], F32R, tag="ec")
            nc.vector.tensor_copy(ec, et_stage)
            return ec

        def phase_mm2_fin(s, lt, ec, rz):
            o_sb = osb_pool.tile([O, T], F32, tag="osb")
            for h in range(NH):
                op = o_ps.tile([O, HW], F32, tag="ops")
                for c in range(ND):
                    nc.tensor.matmul(
                        op,
                        lhsT=ec[:, c, :],
                        rhs=lt[:, c, h * HW : (h + 1) * HW],
                        start=(c == 0),
                        stop=(c == ND - 1),
                    )
                # out = op * (1/Z) during PSUM->SBUF copy
                nc.scalar.activation(
                    out=o_sb[:, h * HW : (h + 1) * HW],
                    in_=op,
                    func=mybir.ActivationFunctionType.Copy,
                    scale=rz,
                )
            nc.sync.dma_start(
                out=out[s].rearrange("(o t) -> o t", o=O), in_=o_sb
            )

        # software-pipelined schedule over the per-core samples
        lt0 = phase_load_transpose(0)
        st0 = phase_mm1(0, lt0)
        e0, rz0 = phase_softmax(0, st0)
        prev = (lt0, e0, rz0)
        for s in range(1, b_per):
            lt_n = phase_load_transpose(s)
            st_n = phase_mm1(s, lt_n)
            # finish previous sample while this one's softmax runs
            lt_p, e_p, rz_p = prev
            ec_p = phase_et(s - 1, e_p)
            phase_mm2_fin(s - 1, lt_p, ec_p, rz_p)
            e_n, rz_n = phase_softmax(s, st_n)
            prev = (lt_n, e_n, rz_n)
        lt_l, e_l, rz_l = prev
        ec_l = phase_et(b_per - 1, e_l)
        phase_mm2_fin(b_per - 1, lt_l, ec_l, rz_l)

    nc.compile()
    return nc


_NC = None
TRACE = False
LAST_RESULT = None


def _get_nc():
    global _NC
    if _NC is None:
        _NC = build_nc()
    return _NC


def kernel(logits, decision, W, b):
    """Full-input entry point: shards batch over 8 cores, returns (16, 10240)."""
    global LAST_RESULT
    logits = np.asarray(logits, dtype=np.float32)
    W = np.asarray(W, dtype=np.float32)
    b = np.asarray(b, dtype=np.float32)
    nc = _get_nc()
    bp = B_FULL // N_CORES
    in_maps = [
        {"logits": np.ascontiguousarray(logits[i * bp : (i + 1) * bp]), "W": W, "b": b}
        for i in range(N_CORES)
    ]
    res = run_bass_kernel_spmd(nc, in_maps, core_ids=list(range(N_CORES)), trace=TRACE)
    LAST_RESULT = res
    return np.concatenate([res.results[i]["out"] for i in range(N_CORES)], axis=0)
